# revision 63
# baseline (speedup 1.0000x reference)
"""Trainium2 Bass kernel for nn_DynamicFiltering (v2).

Computation (per batch b):
  y  = LeakyReLU(conv2d(x_t, w1, b1), 0.2)        per frame t
  ker = conv2d(y, w2, b2)                          (t, 9, h, w)
  ker = ker - mean_K(ker) + 1/45                   per-pixel over K = 45
  out[c,h,w] = sum_{t,k1,k2} x_edge[c,t,h+k1-1,w+k2-1] * ker[t,k1,k2][h,w]

Sharding: 8 cores = 2 batches x 4 H-slabs of 32 rows.

Structure (v2, vs the 234us bf16 baseline):
  - all 16-bit data is fp16 (same speed as bf16, ~8x less quant error)
  - conv1 leaky relu is a single Act Prelu(alpha=0.2) drain (verified on
    HW: Prelu honors alpha, Lrelu hardcodes 0.01) -> GpSimd fully freed
  - pass 2 (lone frame 4) is spatially halved: both partition halves
    carry frame-4 channels over half the rows, same block-diag weights
    as the 2-frame passes -> conv1/conv2 pass-2 matmul cycles halve
  - dynamic-filter products run on DVE in 2x fp16 mode: the di=1 row
    window is served by a separate host copy (xts) so every slice start
    is 4B-aligned
  - only the CENTER (dj=1) pixel-partition x copy is loaded; the dj
    column shift moves to the kernel side: kt2 is partition-shifted by
    +-1 via SBUF-SBUF DMA (2KB/partition vs 40KB for x copies) and the
    PE accumulate uses shifted identity matrices (eye(k=+-1)).  The
    edge-replication terms (q=0 dj=0, q=127 dj=2) multiply the same x
    element as the dj=1 term, so they fold into the dj=1 kernel's edge
    values with one tiny DVE add per edge per pass
  - no tree reduction / pass sums: product pairs (same dj) are added
    once on DVE, then PE matmuls accumulate each pair tile into a
    persistent 4-bank PSUM accumulator, interleaved with the next
    pass's conv matmuls (PSUM: 2 conv1 + 2 conv2 + 4 acc banks = 8)
  - normalization term c*S as in v1 (U chain: sums on DVE in the
    pre-product idle window, shifts/box-sums on GpSimd)
  - output: acc -> fp16 -> 16 DMA-xbar transposes -> DRAM (host casts
    to fp32); no PE transposes, no fp32 identity
  - startup: w1 + pass-0 conv input bands issued first on the sync
    HWDGE ring; all other constants + pass-1/2 inputs on the act ring
"""

import numpy as np

DIM = 64
T = 5
H = 128
W = 128
SLAB = 32          # output rows per core
NCORES = 8
GH = 36            # conv grid rows, passes 0/1: slab + 2*2 halo
GH2 = 20           # conv grid rows, pass 2 halves: 16 + 2*2 halo
GW = 130           # conv grid cols: W + 2
FR = 34            # filter rows: slab + 2 halo
NPASS = 3

_PROGRAM_CACHE = {}

C1_CHUNKS = [(1 + 4 * i, 4) for i in range(8)] + [(33, 2)]
C2_CHUNKS = [(2 + 4 * i, 4) for i in range(8)]
C1_CHUNKS2 = [(1 + 4 * i, 4) for i in range(4)] + [(17, 2)]
C2_CHUNKS2 = [(2 + 4 * i, 4) for i in range(4)]


def _build_program():
    import concourse.bacc as bacc
    import concourse.mybir as mybir
    from concourse.tile import TileContext

    f32 = mybir.dt.float32
    f16 = mybir.dt.float16
    u16 = mybir.dt.uint16
    Act = mybir.ActivationFunctionType
    Alu = mybir.AluOpType
    Ax = mybir.AxisListType

    nc = bacc.Bacc("TRN2", debug=False)

    xc01_d = nc.dram_tensor("xc01", [2, 128, GH, GW], f16, kind="ExternalInput").ap()
    xc2_d = nc.dram_tensor("xc2", [128, GH2, GW], f16, kind="ExternalInput").ap()
    xt_d = nc.dram_tensor("xt", [W, T, DIM, FR], f16, kind="ExternalInput").ap()
    xts_d = nc.dram_tensor("xts", [W, T, DIM, SLAB], f16, kind="ExternalInput").ap()
    sm_d = nc.dram_tensor("sm", [128, 2, 128], f16, kind="ExternalInput").ap()
    em_d = nc.dram_tensor("em", [128, 3], f32, kind="ExternalInput").ap()
    w1_d = nc.dram_tensor("w1", [128, 9, 128], f16, kind="ExternalInput").ap()
    w2_d = nc.dram_tensor("w2", [128, 9, 18], f16, kind="ExternalInput").ap()
    b1_d = nc.dram_tensor("b1r", [128, 1], f32, kind="ExternalInput").ap()
    b2_d = nc.dram_tensor("b2r", [18, 1], f32, kind="ExternalInput").ap()
    ym_d = nc.dram_tensor("ym", [128, 2], f32, kind="ExternalInput").ap()
    ym2_d = nc.dram_tensor("ym2", [128, 2], f32, kind="ExternalInput").ap()
    idf_d = nc.dram_tensor("idf", [128, 128], f16, kind="ExternalInput").ap()
    # out[b_, a, q] = acc[q, 128*a + b_]; the host unscrambles (c r) =
    # 128*a + b_ back to [c, r] (one contiguous DMA instead of 16)
    out_d = nc.dram_tensor("out", [128, 16, W], f16, kind="ExternalOutput").ap()

    with TileContext(nc) as tc:
        with (
            tc.tile_pool(name="consts", bufs=1) as cpool,
            tc.tile_pool(name="xtp", bufs=1) as xtp,
            tc.tile_pool(name="xcp", bufs=2) as xcp,
            tc.tile_pool(name="xc2p", bufs=1) as xc2p,
            tc.tile_pool(name="yp", bufs=3) as yp,
            tc.tile_pool(name="ksh", bufs=2) as kshp,
            tc.tile_pool(name="kst", bufs=1) as kstp,
            tc.tile_pool(name="kta", bufs=2) as ktap,
            tc.tile_pool(name="ktp", bufs=1) as ktp,
            tc.tile_pool(name="up", bufs=1) as up,
            tc.tile_pool(name="tp", bufs=18) as tp,
            tc.tile_pool(name="obp", bufs=2) as obp,
        ):
            # ---- startup DMAs: sync ring carries only what gates the ----
            # ---- first conv1 matmuls (w1 + pass-0 input bands)        ----
            w1_sb = cpool.tile([128, 9, 128], f16)
            nc.sync.dma_start(out=w1_sb, in_=w1_d)

            def load_xc01(p, eng):
                t = xcp.tile([128, GH, GW], f16, tag="xc")
                for r0b, r1b in ((0, 8), (8, 16), (16, 24), (24, 32), (32, 36)):
                    eng.dma_start(out=t[:, r0b:r1b], in_=xc01_d[p, :, r0b:r1b])
                return t

            xc_p0 = load_xc01(0, nc.sync)

            # filter inputs on the sync HWDGE ring AFTER the pass-0 conv
            # bands: per-ring FIFO means the bands drain at full
            # bandwidth first (on the SWDGE ring their huge descriptors
            # monopolized the SDMA engines and stalled conv1 pass 0)
            xt1 = xtp.tile([W, T, DIM, FR], f16, name="xt1")
            xts1 = xtp.tile([W, T, DIM, SLAB], f16, name="xts1")
            nc.sync.dma_start(out=xt1, in_=xt_d)
            nc.sync.dma_start(out=xts1, in_=xts_d)

            # act HWDGE ring: everything else, in need-order
            b1_sb = cpool.tile([128, 1], f32)
            nc.scalar.dma_start(out=b1_sb, in_=b1_d)
            w2_sb = cpool.tile([128, 9, 18], f16)
            nc.scalar.dma_start(out=w2_sb, in_=w2_d)
            b2_sb = cpool.tile([18, 1], f32)
            nc.scalar.dma_start(out=b2_sb, in_=b2_d)
            ym_sb = cpool.tile([128, 2], f32)
            nc.scalar.dma_start(out=ym_sb, in_=ym_d)
            ym2_sb = cpool.tile([128, 2], f32)
            nc.scalar.dma_start(out=ym2_sb, in_=ym2_d)
            idf_sb = cpool.tile([128, 128], f16)
            nc.scalar.dma_start(out=idf_sb, in_=idf_d)
            sm_sb = cpool.tile([128, 2, 128], f16)
            nc.scalar.dma_start(out=sm_sb, in_=sm_d)
            em_sb = cpool.tile([128, 3], f32)
            nc.scalar.dma_start(out=em_sb, in_=em_d)
            # pass-1/2 conv inputs also on the sync ring: DMAs on the act
            # ring would block the conv1 Prelu drains behind their
            # completions (act-queue FIFO) and stall conv1 on PSUM reuse
            xc_p1 = load_xc01(1, nc.sync)
            xc_p2 = xc2p.tile([128, GH2, GW], f16)
            for r0b, r1b in ((0, 8), (8, 16), (16, 20)):
                nc.sync.dma_start(out=xc_p2[:, r0b:r1b], in_=xc2_d[:, r0b:r1b])

            # conv2 -> kernel staging (ti on partitions)
            ker_st = kstp.tile([32, SLAB, W], f16)
            nc.gpsimd.memset(ker_st.bitcast(u16), 0)
            # warm the Q7 tensor-op ucode (~6us LIBRARY_RELOAD) off the
            # critical path; the writes are zeros into staging rows that
            # conv2 either overwrites or the transpose reads as zero
            nc.gpsimd.tensor_copy(ker_st[:, 0:1, 0:64], ker_st[:, 0:1, 64:128])
            nc.gpsimd.tensor_tensor(ker_st[:, 0:1, 0:64], ker_st[:, 0:1, 0:64],
                                    ker_st[:, 0:1, 64:128], Alu.add)

            # y tiles pre-allocated; edge cols zeroed up front on gpsimd
            y_t = [yp.tile([128, GH, GW], f16, name=f"y{p}", tag="y")
                   for p in range(3)]
            for p in range(3):
                nr = 34 if p < 2 else 18
                nc.gpsimd.memset(y_t[p][:, 1:1 + nr, 0:1].bitcast(u16), 0)
                nc.gpsimd.memset(y_t[p][:, 1:1 + nr, 129:130].bitcast(u16), 0)

            # per-pixel kernels, half-major: kt2[p][q, h, ti, r] with
            # h = 16-row kernel half (enables per-half pipelining and
            # contiguous shift DMAs)
            kt2 = [ktp.tile([W, 2, 18, 16], f16, name=f"kt2_{p}")
                   for p in range(NPASS)]

            # U chain on DVE in the idle window before the first products
            # (xt arrives ~30us, first kernels ~50us).  ub = 3-row box sum
            # of U = sum_t x_t; the q-direction box happens at accumulate
            # time via the same shifted-identity trick as the products,
            # with shifted copies of the tiny coefficient c instead of a
            # (slow) partition-shifted copy of U
            u_c = up.tile([W, DIM, FR], f16, name="u_c")
            ub = up.tile([W, DIM, SLAB], f16, name="ub")
            nc.vector.tensor_tensor(u_c, xt1[:, 0], xt1[:, 1], Alu.add)
            for t_i in (2, 3, 4):
                nc.vector.tensor_tensor(u_c, u_c, xt1[:, t_i], Alu.add)
            nc.vector.tensor_tensor(ub, u_c[:, :, 0:SLAB],
                                    u_c[:, :, 1:SLAB + 1], Alu.add)
            nc.vector.tensor_tensor(ub, ub, u_c[:, :, 2:SLAB + 2], Alu.add)

            r_p = [ktp.tile([W, SLAB], f32, name=f"r{p}") for p in range(NPASS)]

            with (
                tc.tile_pool(name="ps1", bufs=2, space="PSUM") as ps1p,
                tc.tile_pool(name="ps2", bufs=2, space="PSUM") as ps2p,
                tc.tile_pool(name="acc", bufs=1, space="PSUM") as accp,
            ):
                # acc layout [q, (h, c, rr)]: half-major so a half tile
                # accumulates into its own 2 PSUM banks
                acc = accp.tile([W, DIM * SLAB], f32)
                pending = []          # (tile, dj, h) awaiting accumulate
                acc_first = [True, True]

                def acc_mm(tile, dj, h, last):
                    # dj=1: plain identity; dj=0/2: shifted identity
                    # applies the +-1 pixel-column shift of the patches
                    lhs = (sm_sb[:, 0, :], idf_sb, sm_sb[:, 1, :])[dj]
                    fl = tile.rearrange("q c r -> q (c r)")
                    for cc in range(2):
                        sl = slice(1024 * h + 512 * cc,
                                   1024 * h + 512 * (cc + 1))
                        nc.tensor.matmul(acc[:, sl], lhsT=lhs,
                                         rhs=fl[:, 512 * cc:512 * (cc + 1)],
                                         start=acc_first[h], stop=last)
                    acc_first[h] = False

                def drain_acc(n):
                    for _ in range(min(n, len(pending))):
                        tile, dj, h = pending.pop(0)
                        acc_mm(tile, dj, h, False)

                def final_drain():
                    last_of = {}
                    for i, (_, _, h) in enumerate(pending):
                        last_of[h] = i
                    for i, (tile, dj, h) in enumerate(pending):
                        acc_mm(tile, dj, h, last_of[h] == i)
                    pending.clear()

                for p in range(NPASS):
                    xc_f = (xc_p0, xc_p1, xc_p2)[p]
                    y_f = y_t[p]
                    c1 = C1_CHUNKS if p < 2 else C1_CHUNKS2
                    c2 = C2_CHUNKS if p < 2 else C2_CHUNKS2

                    ymm = ym_sb if p < 2 else ym2_sb
                    hrow = 34 if p < 2 else 18

                    def conv1_chunk(ci):
                        g0, nr = c1[ci]
                        ps = ps1p.tile([128, 4, W], f32, tag="ps1")
                        for idx in range(9):
                            di, dj = divmod(idx, 3)
                            rhs = xc_f[:, g0 + di - 1:g0 + di - 1 + nr,
                                       dj:dj + W]
                            nc.tensor.matmul(
                                ps[:, :nr, :], lhsT=w1_sb[:, idx, :], rhs=rhs,
                                start=(idx == 0), stop=(idx == 8))
                        nc.scalar.activation(y_f[:, g0:g0 + nr, 1:129],
                                             ps[:, :nr], Act.Prelu,
                                             bias=b1_sb, scale=1.0, alpha=0.2)
                        # conv2 zero-pads rows outside the image: kill the
                        # y halo row as soon as its chunk drains
                        if ci == 0:
                            nc.scalar.activation(y_f[:, 1:2, 1:129],
                                                 y_f[:, 1:2, 1:129],
                                                 Act.Copy, scale=ymm[:, 0:1])
                        if ci == len(c1) - 1:
                            nc.scalar.activation(
                                y_f[:, hrow:hrow + 1, 1:129],
                                y_f[:, hrow:hrow + 1, 1:129],
                                Act.Copy, scale=ymm[:, 1:2])
                        if p >= 1 and ci >= 1:
                            drain_acc(1)

                    def half_pipeline(p, h, ktA):
                        # repack ktA[q, r, ti] -> kt2[p][q, h, ti, r] on
                        # DVE (288 elems -> ~0.4us, and no cross-engine
                        # latency before the products that follow)
                        if p < 2:
                            nc.vector.tensor_copy(
                                kt2[p][:, h, :, :],
                                ktA[:, :, 0:18].rearrange("q r t -> q t r"))
                        else:
                            # spatial halves arrive as taps 0-8 / 9-17
                            nc.vector.tensor_copy(
                                kt2[2][:, h, 0:9, :],
                                ktA[:, :, 9 * h:9 * h + 9]
                                .rearrange("q r t -> q t r"))
                        nt = 18 if p < 2 else 9
                        # kernel sums for the normalization coefficient
                        # (must read the PRE-merge values)
                        nc.vector.tensor_reduce(
                            r_p[p][:, 16 * h:16 * h + 16],
                            kt2[p][:, h, 0:nt, :].rearrange("q t r -> q r t"),
                            axis=Ax.X, op=Alu.add)
                        # fold the edge-replicated dj=0 (q=0) / dj=2
                        # (q=127) terms into the dj=1 kernel: they multiply
                        # the same x element as the dj=1 term there.
                        # Engines can't start mid-partition, so mask with
                        # a per-partition one-hot
                        ev = kt2[p][:, h, 0:nt, :].rearrange(
                            "q (a b) r -> q a b r", b=3)
                        nc.vector.scalar_tensor_tensor(
                            ev[:, :, 1, :], ev[:, :, 0, :], em_sb[:, 0:1],
                            ev[:, :, 1, :], Alu.mult, Alu.add)
                        nc.vector.scalar_tensor_tensor(
                            ev[:, :, 1, :], ev[:, :, 2, :], em_sb[:, 1:2],
                            ev[:, :, 1, :], Alu.mult, Alu.add)
                        # partition-shifted kernel copies for dj=0 / dj=2
                        # (products run at the source pixel; the PE
                        # accumulate shifts them into place)
                        # (rows ktp_t[127] / ktm_t[0] are killed by the
                        # zero row of the shifted identities, but must
                        # hold FINITE values: 0 * NaN would poison PSUM)
                        ktp_t = kshp.tile([W, 18, 16], f16, tag="kp")
                        ktm_t = kshp.tile([W, 18, 16], f16, tag="km")
                        nc.gpsimd.dma_start(out=ktp_t[0:127],
                                            in_=kt2[p][1:128, h, :, :])
                        nc.gpsimd.dma_start(out=ktp_t[127:128],
                                            in_=kt2[p][127:128, h, :, :])
                        nc.gpsimd.dma_start(out=ktm_t[1:128],
                                            in_=kt2[p][0:127, h, :, :])
                        nc.gpsimd.dma_start(out=ktm_t[0:1],
                                            in_=kt2[p][0:1, h, :, :])

                        # products: pairs (same dj) -> one DVE add -> PE
                        # accumulate (drained interleaved with conv)
                        # di order (0,2,1): the di=1 groups read xts1,
                        # which lands last on the sync ring
                        if p < 2:
                            groups = [[(2 * p + fi, fi * 9 + 3 * di + dj,
                                        di, dj) for fi in (0, 1)]
                                      for dj in (1, 0, 2) for di in (0, 2, 1)]
                        else:
                            groups = []
                            for dj in (1, 0, 2):
                                terms = [(4, 3 * di + dj, di, dj)
                                         for di in (0, 2, 1)]
                                groups += [terms[0:2], terms[2:3]]
                        for gi, g in enumerate(groups):
                            prods = []
                            for (f, ti, di, dj) in g:
                                if dj == 1:
                                    kb = kt2[p][:, h, ti, :]
                                else:
                                    kb = (ktp_t if dj == 0 else ktm_t)[:, ti, :]
                                kb = kb.unsqueeze(1)\
                                    .broadcast_to((W, DIM, 16))
                                if di == 1:
                                    xs = xts1[:, f, :, 16 * h:16 * h + 16]
                                else:
                                    xs = xt1[:, f, :,
                                             di + 16 * h:di + 16 * h + 16]
                                prod = tp.tile([W, DIM, 16], f16, tag="ts")
                                nc.vector.tensor_tensor(prod, xs, kb,
                                                        Alu.mult)
                                prods.append(prod)
                            if len(prods) == 2:
                                # last pass: pair-adds ride on the idle
                                # gpsimd, shortening the serial DVE tail
                                eng = nc.gpsimd if p == 2 else nc.vector
                                eng.tensor_tensor(prods[0], prods[0],
                                                  prods[1], Alu.add)
                            pending.append((prods[0], g[0][3], h))
                            # last pass: no later conv to interleave into
                            if p == 2 and gi >= 1:
                                drain_acc(1)

                    def emit_transpose(lo):
                        ktA = ktap.tile([W, 16, 32], f16, tag="ktA")
                        nc.scalar.dma_start_transpose(
                            out=ktA,
                            in_=ker_st[:, lo:lo + 16, :]
                            .rearrange("ti r q -> ti (r q)"))
                        return ktA

                    def conv2_chunk(ci):
                        g0, nr = c2[ci]
                        ps2 = ps2p.tile([18, 4, W], f32, tag="ps2")
                        for idx in range(9):
                            di, dj = divmod(idx, 3)
                            rhs = y_f[:, g0 + di - 1:g0 + di - 1 + nr,
                                      dj:dj + W]
                            nc.tensor.matmul(
                                ps2[:, :nr, :], lhsT=w2_sb[:, idx, :], rhs=rhs,
                                start=(idx == 0), stop=(idx == 8))
                        nc.scalar.activation(ker_st[0:18, g0 - 2:g0 - 2 + nr, :],
                                             ps2[:, :nr], Act.Identity,
                                             bias=b2_sb, scale=1.0)
                        if p > 0:
                            drain_acc(2 if ci % 2 == 1 else 1)

                    # interleave: conv2 chunk k only needs conv1 chunks
                    # <= k+1, so each 16-row kernel half completes (and
                    # its products start) as early as possible
                    conv1_chunk(0)
                    conv1_chunk(1)
                    for k in range(len(c2)):
                        if k + 2 < len(c1):
                            conv1_chunk(k + 2)
                        conv2_chunk(k)
                        if p < 2 and k == 3:
                            half_pipeline(p, 0, emit_transpose(0))
                    if p < 2:
                        half_pipeline(p, 1, emit_transpose(16))
                    else:
                        ktA2 = emit_transpose(0)
                        half_pipeline(2, 0, ktA2)
                        half_pipeline(2, 1, ktA2)

                # --- normalization: c = 1/45 - mean(ker); out += c * S
                # with S = 3x3 box of U.  The q-box comes from the three
                # shifted-identity accumulates; edge replication doubles
                # c at q=0/127 in the center (dj=1) tile ---
                nc.vector.tensor_tensor(r_p[0], r_p[0], r_p[1], Alu.add)
                nc.vector.tensor_tensor(r_p[0], r_p[0], r_p[2], Alu.add)
                c_sb = ktp.tile([W, SLAB], f32, name="c_sb")
                nc.vector.tensor_scalar(c_sb, r_p[0], -1.0 / 45.0, 1.0 / 45.0,
                                        Alu.mult, Alu.add)
                c_bf = ktp.tile([W, SLAB], f16, name="c_bf")
                nc.vector.tensor_copy(c_bf, c_sb)
                c_db = ktp.tile([W, SLAB], f16, name="c_db")
                nc.vector.tensor_scalar_mul(c_db, c_bf, em_sb[:, 2:3])
                c_p = ktp.tile([W, SLAB], f16, name="c_p")
                c_m = ktp.tile([W, SLAB], f16, name="c_m")
                nc.gpsimd.dma_start(out=c_p[0:127], in_=c_bf[1:128])
                nc.gpsimd.dma_start(out=c_p[127:128], in_=c_bf[127:128])
                nc.gpsimd.dma_start(out=c_m[1:128], in_=c_bf[0:127])
                nc.gpsimd.dma_start(out=c_m[0:1], in_=c_bf[0:1])
                # cs products on gpsimd: they only need c (ready while
                # the DVE is still on pass-2 products) and finish in its
                # shadow
                for (cc_, djc) in ((c_db, 1), (c_p, 0), (c_m, 2)):
                    for h in range(2):
                        cs = tp.tile([W, DIM, 16], f16, tag="ts")
                        nc.gpsimd.tensor_tensor(
                            cs, ub[:, :, 16 * h:16 * h + 16],
                            cc_[:, 16 * h:16 * h + 16].unsqueeze(1)
                            .broadcast_to((W, DIM, 16)),
                            Alu.mult)
                        pending.append((cs, djc, h))
                        drain_acc(1)

                final_drain()

                # drain acc -> fp16, then DMA-xbar transposes to DRAM
                acc_sb = ktp.tile([W, DIM * SLAB], f16, name="acc_sb")
                for cc in range(2):
                    sl = slice(512 * cc, 512 * (cc + 1))
                    nc.scalar.activation(acc_sb[:, sl], acc[:, sl],
                                         Act.Copy, scale=1.0)
                for cc in range(2, 4):
                    sl = slice(512 * cc, 512 * (cc + 1))
                    nc.vector.tensor_copy(acc_sb[:, sl], acc[:, sl])
                for h in range(2):
                    ob = obp.tile([128, 8, W], f16, tag="ob")
                    nc.sync.dma_start_transpose(
                        out=ob, in_=acc_sb[:, 1024 * h:1024 * (h + 1)])
                    nc.sync.dma_start(out=out_d[:, 8 * h:8 * h + 8, :],
                                      in_=ob)

    return nc


def _get_program():
    if "nc" not in _PROGRAM_CACHE:
        nc = _build_program()
        nc.finalize()
        _PROGRAM_CACHE["nc"] = nc
    return _PROGRAM_CACHE["nc"]


def _host_prep(x, w1, b1, w2, b2):
    """Build the 8 per-core input maps from full inputs."""
    x = np.asarray(x, dtype=np.float32)
    w1 = np.asarray(w1, dtype=np.float32)
    b1 = np.asarray(b1, dtype=np.float32)
    w2 = np.asarray(w2, dtype=np.float32)
    b2 = np.asarray(b2, dtype=np.float32)
    f16 = np.float16

    # block-diagonal packed weights: passes 0/1 = 2 frames, pass 2 = the
    # two spatial halves of frame 4 -> identical weight matrices
    w1t = w1.transpose(1, 2, 3, 0).reshape(DIM, 9, DIM)   # [ci, tap, o]
    w2t = w2.transpose(1, 2, 3, 0).reshape(DIM, 9, 9)
    w1a = np.zeros((128, 9, 128), np.float32)
    w1a[0:64, :, 0:64] = w1t
    w1a[64:128, :, 64:128] = w1t
    w2a = np.zeros((128, 9, 18), np.float32)
    w2a[0:64, :, 0:9] = w2t
    w2a[64:128, :, 9:18] = w2t

    b1r = np.concatenate([b1, b1]).reshape(128, 1).astype(np.float32)
    b2r = np.concatenate([b2, b2]).reshape(18, 1).astype(np.float32)
    idf = np.eye(128, dtype=f16)
    w1a = w1a.astype(f16)
    w2a = w2a.astype(f16)

    in_maps = []
    for core in range(NCORES):
        b, s = divmod(core, 4)
        r0 = s * SLAB
        # passes 0/1 conv input: frames (2p, 2p+1) on the partition
        # halves, x rows r0-2 .. r0+33 zero padded, cols -1..128 zero
        xc01 = np.zeros((2, 128, GH, GW), np.float32)
        lo = max(0, r0 - 2)
        hi = min(H, r0 + 34)
        for p in range(2):
            for f in range(2):
                t = 2 * p + f
                xc01[p, f * 64:(f + 1) * 64,
                     lo - (r0 - 2):hi - (r0 - 2), 1:129] = x[b, :, t, lo:hi, :]
        # pass 2: frame 4 split into two 16-row halves on the partition
        # halves (plus conv halo)
        xc2 = np.zeros((128, GH2, GW), np.float32)
        for h2 in range(2):
            bx = r0 - 2 if h2 == 0 else r0 + 14
            lo2 = max(0, bx)
            hi2 = min(H, bx + GH2)
            xc2[h2 * 64:(h2 + 1) * 64, lo2 - bx:hi2 - bx, 1:129] = \
                x[b, :, 4, lo2:hi2, :]
        # filter input, pixel-partition, center (dj=1) copy only; xts =
        # the r0-based row window so di=1 product slices start 4B-aligned
        rows = np.clip(np.arange(r0 - 1, r0 + 33), 0, H - 1)
        xt = x[b][:, :, rows, :].transpose(3, 1, 0, 2)          # (w,t,c,34)
        xts = x[b][:, :, r0:r0 + 32, :].transpose(3, 1, 0, 2)   # (w,t,c,32)
        # shifted identities for the dj=0/dj=2 accumulates
        sm = np.zeros((128, 2, 128), np.float32)
        sm[0:127, 0, :] = np.eye(128, dtype=np.float32)[1:128]   # m = p+1
        sm[1:128, 1, :] = np.eye(128, dtype=np.float32)[0:127]   # m = p-1
        em = np.zeros((128, 3), np.float32)
        em[0, 0] = 1.0      # q=0 edge (dj=0 term folds into dj=1)
        em[127, 1] = 1.0    # q=127 edge (dj=2 term folds into dj=1)
        em[:, 2] = 1.0      # edge-doubling mask for the c*S center tile
        em[0, 2] = 2.0
        em[127, 2] = 2.0
        # conv2 zero-pad masks for y rows outside the image
        ym = np.ones((128, 2), np.float32)
        if s == 0:
            ym[:, 0] = 0.0
        if s == 3:
            ym[:, 1] = 0.0
        ym2 = np.ones((128, 2), np.float32)
        if s == 0:
            ym2[0:64, 0] = 0.0
        if s == 3:
            ym2[64:128, 1] = 0.0
        in_maps.append({
            "xc01": xc01.astype(f16), "xc2": xc2.astype(f16),
            "xt": xt.astype(f16), "xts": xts.astype(f16),
            "w1": w1a, "w2": w2a, "b1r": b1r, "b2r": b2r,
            "ym": ym, "ym2": ym2, "idf": idf, "sm": sm.astype(f16),
            "em": em,
        })
    return in_maps


def kernel(x, w1, b1, w2, b2):
    from concourse.bass_utils import run_bass_kernel_spmd

    nc = _get_program()
    in_maps = _host_prep(x, w1, b1, w2, b2)
    res = run_bass_kernel_spmd(nc, in_maps, list(range(NCORES)))
    out = np.zeros((2, DIM, H, W), dtype=np.float32)
    for core in range(NCORES):
        b, s = divmod(core, 4)
        # device layout: o[b_, a, q] = acc[q, flat = 128*a + b_] with
        # flat = 1024*h + 16*c + rr and out row r = 16*h + rr
        o = res.results[core]["out"].astype(np.float32)
        o = o.transpose(1, 0, 2).reshape(2, DIM, 16, W)
        o = o.transpose(1, 0, 2, 3).reshape(DIM, SLAB, W)
        out[b, :, s * SLAB:(s + 1) * SLAB, :] = o
    return out


# revision 66
# speedup vs baseline: 1.1250x; 1.1250x over previous
"""Trainium2 Bass kernel for nn_DynamicFiltering (v2).

Computation (per batch b):
  y  = LeakyReLU(conv2d(x_t, w1, b1), 0.2)        per frame t
  ker = conv2d(y, w2, b2)                          (t, 9, h, w)
  ker = ker - mean_K(ker) + 1/45                   per-pixel over K = 45
  out[c,h,w] = sum_{t,k1,k2} x_edge[c,t,h+k1-1,w+k2-1] * ker[t,k1,k2][h,w]

Sharding: 8 cores = 2 batches x 4 H-slabs of 32 rows.

Structure (v2, vs the 234us bf16 baseline):
  - all 16-bit data is fp16 (same speed as bf16, ~8x less quant error)
  - conv1 leaky relu is a single Act Prelu(alpha=0.2) drain (verified on
    HW: Prelu honors alpha, Lrelu hardcodes 0.01) -> GpSimd fully freed
  - pass 2 (lone frame 4) is spatially halved: both partition halves
    carry frame-4 channels over half the rows, same block-diag weights
    as the 2-frame passes -> conv1/conv2 pass-2 matmul cycles halve
  - dynamic-filter products run on DVE in 2x fp16 mode: the di=1 row
    window is served by a separate host copy (xts) so every slice start
    is 4B-aligned
  - only the CENTER (dj=1) pixel-partition x copy is loaded; the dj
    column shift moves to the kernel side: kt2 is partition-shifted by
    +-1 via SBUF-SBUF DMA (2KB/partition vs 40KB for x copies) and the
    PE accumulate uses shifted identity matrices (eye(k=+-1)).  The
    edge-replication terms (q=0 dj=0, q=127 dj=2) multiply the same x
    element as the dj=1 term, so they fold into the dj=1 kernel's edge
    values with one tiny DVE add per edge per pass
  - no tree reduction / pass sums: product pairs (same dj) are added
    once on DVE, then PE matmuls accumulate each pair tile into a
    persistent 4-bank PSUM accumulator, interleaved with the next
    pass's conv matmuls (PSUM: 2 conv1 + 2 conv2 + 4 acc banks = 8)
  - normalization term c*S as in v1 (U chain: sums on DVE in the
    pre-product idle window, shifts/box-sums on GpSimd)
  - output: acc -> fp16 -> 16 DMA-xbar transposes -> DRAM (host casts
    to fp32); no PE transposes, no fp32 identity
  - startup: w1 + pass-0 conv input bands issued first on the sync
    HWDGE ring; all other constants + pass-1/2 inputs on the act ring
"""

import numpy as np

DIM = 64
T = 5
H = 128
W = 128
SLAB = 32          # output rows per core
NCORES = 8
GH = 36            # conv grid rows, passes 0/1: slab + 2*2 halo
GH2 = 20           # conv grid rows, pass 2 halves: 16 + 2*2 halo
GW = 130           # conv grid cols: W + 2
FR = 34            # filter rows: slab + 2 halo
NPASS = 3

_PROGRAM_CACHE = {}

C1_CHUNKS = [(1 + 4 * i, 4) for i in range(8)] + [(33, 2)]
C2_CHUNKS = [(2 + 4 * i, 4) for i in range(8)]
C1_CHUNKS2 = [(1 + 4 * i, 4) for i in range(4)] + [(17, 2)]
C2_CHUNKS2 = [(2 + 4 * i, 4) for i in range(4)]


def _build_program():
    import concourse.bacc as bacc
    import concourse.mybir as mybir
    from concourse.tile import TileContext

    f32 = mybir.dt.float32
    f16 = mybir.dt.float16
    u16 = mybir.dt.uint16
    Act = mybir.ActivationFunctionType
    Alu = mybir.AluOpType
    Ax = mybir.AxisListType

    nc = bacc.Bacc("TRN2", debug=False)

    xc01_d = nc.dram_tensor("xc01", [2, 128, GH, GW], f16, kind="ExternalInput").ap()
    xc2_d = nc.dram_tensor("xc2", [128, GH2, GW], f16, kind="ExternalInput").ap()
    xt_d = nc.dram_tensor("xt", [W, T, DIM, FR], f16, kind="ExternalInput").ap()
    xts_d = nc.dram_tensor("xts", [W, T, DIM, SLAB], f16, kind="ExternalInput").ap()
    sm_d = nc.dram_tensor("sm", [128, 2, 128], f16, kind="ExternalInput").ap()
    em_d = nc.dram_tensor("em", [128, 3], f32, kind="ExternalInput").ap()
    w1_d = nc.dram_tensor("w1", [128, 9, 128], f16, kind="ExternalInput").ap()
    w2_d = nc.dram_tensor("w2", [128, 9, 18], f16, kind="ExternalInput").ap()
    b1_d = nc.dram_tensor("b1r", [128, 1], f32, kind="ExternalInput").ap()
    b2_d = nc.dram_tensor("b2r", [18, 1], f32, kind="ExternalInput").ap()
    ym_d = nc.dram_tensor("ym", [128, 2], f32, kind="ExternalInput").ap()
    ym2_d = nc.dram_tensor("ym2", [128, 2], f32, kind="ExternalInput").ap()
    idf_d = nc.dram_tensor("idf", [128, 128], f16, kind="ExternalInput").ap()
    # out[b_, a, q] = acc[q, 128*a + b_]; the host unscrambles (c r) =
    # 128*a + b_ back to [c, r] (one contiguous DMA instead of 16)
    out_d = nc.dram_tensor("out", [128, 16, W], f16, kind="ExternalOutput").ap()

    with TileContext(nc) as tc:
        with (
            tc.tile_pool(name="consts", bufs=1) as cpool,
            tc.tile_pool(name="xtp", bufs=1) as xtp,
            tc.tile_pool(name="xcp", bufs=2) as xcp,
            tc.tile_pool(name="xc2p", bufs=1) as xc2p,
            tc.tile_pool(name="yp", bufs=3) as yp,
            tc.tile_pool(name="ksh", bufs=2) as kshp,
            tc.tile_pool(name="kst", bufs=1) as kstp,
            tc.tile_pool(name="kta", bufs=2) as ktap,
            tc.tile_pool(name="ktp", bufs=1) as ktp,
            tc.tile_pool(name="up", bufs=1) as up,
            tc.tile_pool(name="tp", bufs=18) as tp,
            tc.tile_pool(name="obp", bufs=2) as obp,
        ):
            # ---- startup DMAs: sync ring carries only what gates the ----
            # ---- first conv1 matmuls (w1 + pass-0 input bands)        ----
            w1_sb = cpool.tile([128, 9, 128], f16)
            nc.sync.dma_start(out=w1_sb, in_=w1_d)

            def load_xc01(p, eng):
                t = xcp.tile([128, GH, GW], f16, tag="xc")
                for r0b, r1b in ((0, 8), (8, 16), (16, 24), (24, 32), (32, 36)):
                    eng.dma_start(out=t[:, r0b:r1b], in_=xc01_d[p, :, r0b:r1b])
                return t

            xc_p0 = load_xc01(0, nc.sync)

            # filter inputs on the sync HWDGE ring AFTER the pass-0 conv
            # bands: per-ring FIFO means the bands drain at full
            # bandwidth first (on the SWDGE ring their huge descriptors
            # monopolized the SDMA engines and stalled conv1 pass 0)
            xt1 = xtp.tile([W, T, DIM, FR], f16, name="xt1")
            xts1 = xtp.tile([W, T, DIM, SLAB], f16, name="xts1")
            nc.sync.dma_start(out=xt1, in_=xt_d)
            nc.sync.dma_start(out=xts1, in_=xts_d)

            # act HWDGE ring: everything else, in need-order
            b1_sb = cpool.tile([128, 1], f32)
            nc.scalar.dma_start(out=b1_sb, in_=b1_d)
            w2_sb = cpool.tile([128, 9, 18], f16)
            nc.scalar.dma_start(out=w2_sb, in_=w2_d)
            b2_sb = cpool.tile([18, 1], f32)
            nc.scalar.dma_start(out=b2_sb, in_=b2_d)
            ym_sb = cpool.tile([128, 2], f32)
            nc.scalar.dma_start(out=ym_sb, in_=ym_d)
            ym2_sb = cpool.tile([128, 2], f32)
            nc.scalar.dma_start(out=ym2_sb, in_=ym2_d)
            idf_sb = cpool.tile([128, 128], f16)
            nc.scalar.dma_start(out=idf_sb, in_=idf_d)
            sm_sb = cpool.tile([128, 2, 128], f16)
            nc.scalar.dma_start(out=sm_sb, in_=sm_d)
            em_sb = cpool.tile([128, 3], f32)
            nc.scalar.dma_start(out=em_sb, in_=em_d)
            # pass-1/2 conv inputs also on the sync ring: DMAs on the act
            # ring would block the conv1 Prelu drains behind their
            # completions (act-queue FIFO) and stall conv1 on PSUM reuse
            xc_p1 = load_xc01(1, nc.sync)
            xc_p2 = xc2p.tile([128, GH2, GW], f16)
            for r0b, r1b in ((0, 8), (8, 16), (16, 20)):
                nc.sync.dma_start(out=xc_p2[:, r0b:r1b], in_=xc2_d[:, r0b:r1b])

            # conv2 -> kernel staging (ti on partitions)
            ker_st = kstp.tile([32, SLAB, W], f16)
            nc.gpsimd.memset(ker_st.bitcast(u16), 0)
            # warm the Q7 tensor-op ucode (~6us LIBRARY_RELOAD) off the
            # critical path; the writes are zeros into staging rows that
            # conv2 either overwrites or the transpose reads as zero
            nc.gpsimd.tensor_copy(ker_st[:, 0:1, 0:64], ker_st[:, 0:1, 64:128])
            nc.gpsimd.tensor_tensor(ker_st[:, 0:1, 0:64], ker_st[:, 0:1, 0:64],
                                    ker_st[:, 0:1, 64:128], Alu.add)

            # y tiles pre-allocated; edge cols zeroed up front on gpsimd
            y_t = [yp.tile([128, GH, GW], f16, name=f"y{p}", tag="y")
                   for p in range(3)]
            for p in range(3):
                nr = 34 if p < 2 else 18
                nc.gpsimd.memset(y_t[p][:, 1:1 + nr, 0:1].bitcast(u16), 0)
                nc.gpsimd.memset(y_t[p][:, 1:1 + nr, 129:130].bitcast(u16), 0)

            # per-pixel kernels, half-major: kt2[p][q, h, ti, r] with
            # h = 16-row kernel half (enables per-half pipelining and
            # contiguous shift DMAs)
            kt2 = [ktp.tile([W, 2, 18, 16], f16, name=f"kt2_{p}")
                   for p in range(NPASS)]

            # U chain on DVE in the idle window before the first products
            # (xt arrives ~30us, first kernels ~50us).  ub = 3-row box sum
            # of U = sum_t x_t; the q-direction box happens at accumulate
            # time via the same shifted-identity trick as the products,
            # with shifted copies of the tiny coefficient c instead of a
            # (slow) partition-shifted copy of U
            u_c = up.tile([W, DIM, FR], f16, name="u_c")
            ub = up.tile([W, DIM, SLAB], f16, name="ub")
            nc.vector.tensor_tensor(u_c, xt1[:, 0], xt1[:, 1], Alu.add)
            for t_i in (2, 3, 4):
                nc.vector.tensor_tensor(u_c, u_c, xt1[:, t_i], Alu.add)
            nc.vector.tensor_tensor(ub, u_c[:, :, 0:SLAB],
                                    u_c[:, :, 1:SLAB + 1], Alu.add)
            nc.vector.tensor_tensor(ub, ub, u_c[:, :, 2:SLAB + 2], Alu.add)

            r_p = [ktp.tile([W, SLAB], f32, name=f"r{p}") for p in range(NPASS)]

            with (
                tc.tile_pool(name="ps1", bufs=2, space="PSUM") as ps1p,
                tc.tile_pool(name="ps2", bufs=2, space="PSUM") as ps2p,
                tc.tile_pool(name="acc", bufs=1, space="PSUM") as accp,
            ):
                # acc layout [q, (h, c, rr)]: half-major so a half tile
                # accumulates into its own 2 PSUM banks
                acc = accp.tile([W, DIM * SLAB], f32)
                pending = []          # (tile, dj, h) awaiting accumulate
                acc_first = [True, True]

                def acc_mm(tile, dj, h, last):
                    # dj=1: plain identity; dj=0/2: shifted identity
                    # applies the +-1 pixel-column shift of the patches
                    lhs = (sm_sb[:, 0, :], idf_sb, sm_sb[:, 1, :])[dj]
                    fl = tile.rearrange("q c r -> q (c r)")
                    for cc in range(2):
                        sl = slice(1024 * h + 512 * cc,
                                   1024 * h + 512 * (cc + 1))
                        nc.tensor.matmul(acc[:, sl], lhsT=lhs,
                                         rhs=fl[:, 512 * cc:512 * (cc + 1)],
                                         start=acc_first[h], stop=last)
                    acc_first[h] = False

                def drain_acc(n):
                    for _ in range(min(n, len(pending))):
                        tile, dj, h = pending.pop(0)
                        acc_mm(tile, dj, h, False)

                def final_drain():
                    last_of = {}
                    for i, (_, _, h) in enumerate(pending):
                        last_of[h] = i
                    for i, (tile, dj, h) in enumerate(pending):
                        acc_mm(tile, dj, h, last_of[h] == i)
                    pending.clear()

                for p in range(NPASS):
                    xc_f = (xc_p0, xc_p1, xc_p2)[p]
                    y_f = y_t[p]
                    c1 = C1_CHUNKS if p < 2 else C1_CHUNKS2
                    c2 = C2_CHUNKS if p < 2 else C2_CHUNKS2

                    ymm = ym_sb if p < 2 else ym2_sb
                    hrow = 34 if p < 2 else 18

                    def conv1_chunk(ci):
                        g0, nr = c1[ci]
                        ps = ps1p.tile([128, 4, W], f32, tag="ps1")
                        for idx in range(9):
                            di, dj = divmod(idx, 3)
                            rhs = xc_f[:, g0 + di - 1:g0 + di - 1 + nr,
                                       dj:dj + W]
                            nc.tensor.matmul(
                                ps[:, :nr, :], lhsT=w1_sb[:, idx, :], rhs=rhs,
                                start=(idx == 0), stop=(idx == 8))
                        nc.scalar.activation(y_f[:, g0:g0 + nr, 1:129],
                                             ps[:, :nr], Act.Prelu,
                                             bias=b1_sb, scale=1.0, alpha=0.2)
                        # conv2 zero-pads rows outside the image: kill the
                        # y halo row as soon as its chunk drains
                        if ci == 0:
                            nc.scalar.activation(y_f[:, 1:2, 1:129],
                                                 y_f[:, 1:2, 1:129],
                                                 Act.Copy, scale=ymm[:, 0:1])
                        if ci == len(c1) - 1:
                            nc.scalar.activation(
                                y_f[:, hrow:hrow + 1, 1:129],
                                y_f[:, hrow:hrow + 1, 1:129],
                                Act.Copy, scale=ymm[:, 1:2])
                        if p >= 1 and ci >= 1:
                            drain_acc(1)

                    def half_pipeline(p, h, ktA):
                        # repack ktA[q, r, ti] -> kt2[p][q, h, ti, r] on
                        # DVE (288 elems -> ~0.4us, and no cross-engine
                        # latency before the products that follow)
                        if p < 2:
                            nc.vector.tensor_copy(
                                kt2[p][:, h, :, :],
                                ktA[:, :, 0:18].rearrange("q r t -> q t r"))
                        else:
                            # spatial halves arrive as taps 0-8 / 9-17
                            nc.vector.tensor_copy(
                                kt2[2][:, h, 0:9, :],
                                ktA[:, :, 9 * h:9 * h + 9]
                                .rearrange("q r t -> q t r"))
                        nt = 18 if p < 2 else 9
                        # kernel sums for the normalization coefficient
                        # (must read the PRE-merge values)
                        nc.vector.tensor_reduce(
                            r_p[p][:, 16 * h:16 * h + 16],
                            kt2[p][:, h, 0:nt, :].rearrange("q t r -> q r t"),
                            axis=Ax.X, op=Alu.add)
                        # fold the edge-replicated dj=0 (q=0) / dj=2
                        # (q=127) terms into the dj=1 kernel: they multiply
                        # the same x element as the dj=1 term there.
                        # Engines can't start mid-partition, so mask with
                        # a per-partition one-hot
                        ev = kt2[p][:, h, 0:nt, :].rearrange(
                            "q (a b) r -> q a b r", b=3)
                        nc.vector.scalar_tensor_tensor(
                            ev[:, :, 1, :], ev[:, :, 0, :], em_sb[:, 0:1],
                            ev[:, :, 1, :], Alu.mult, Alu.add)
                        nc.vector.scalar_tensor_tensor(
                            ev[:, :, 1, :], ev[:, :, 2, :], em_sb[:, 1:2],
                            ev[:, :, 1, :], Alu.mult, Alu.add)
                        # partition-shifted kernel copies for dj=0 / dj=2
                        # (products run at the source pixel; the PE
                        # accumulate shifts them into place)
                        # (rows ktp_t[127] / ktm_t[0] are killed by the
                        # zero row of the shifted identities, but must
                        # hold FINITE values: 0 * NaN would poison PSUM)
                        ktp_t = kshp.tile([W, 18, 16], f16, tag="kp")
                        ktm_t = kshp.tile([W, 18, 16], f16, tag="km")
                        nc.gpsimd.dma_start(out=ktp_t[0:127],
                                            in_=kt2[p][1:128, h, :, :])
                        nc.gpsimd.dma_start(out=ktp_t[127:128],
                                            in_=kt2[p][127:128, h, :, :])
                        nc.gpsimd.dma_start(out=ktm_t[1:128],
                                            in_=kt2[p][0:127, h, :, :])
                        nc.gpsimd.dma_start(out=ktm_t[0:1],
                                            in_=kt2[p][0:1, h, :, :])

                        # products: pairs (same dj) -> one DVE add -> PE
                        # accumulate (drained interleaved with conv)
                        # di order (0,2,1): the di=1 groups read xts1,
                        # which lands last on the sync ring
                        if p < 2:
                            groups = [[(2 * p + fi, fi * 9 + 3 * di + dj,
                                        di, dj) for fi in (0, 1)]
                                      for dj in (1, 0, 2) for di in (0, 2, 1)]
                        else:
                            # last pass: no pairing — each product goes
                            # straight to the PE accumulator, shortening
                            # the serial DVE tail (the extra MMs land in
                            # the PE's end-game idle gaps)
                            groups = [[(4, 3 * di + dj, di, dj)]
                                      for dj in (1, 0, 2) for di in (0, 2, 1)]
                        for gi, g in enumerate(groups):
                            prods = []
                            for (f, ti, di, dj) in g:
                                if dj == 1:
                                    kb = kt2[p][:, h, ti, :]
                                else:
                                    kb = (ktp_t if dj == 0 else ktm_t)[:, ti, :]
                                kb = kb.unsqueeze(1)\
                                    .broadcast_to((W, DIM, 16))
                                if di == 1:
                                    xs = xts1[:, f, :, 16 * h:16 * h + 16]
                                else:
                                    xs = xt1[:, f, :,
                                             di + 16 * h:di + 16 * h + 16]
                                prod = tp.tile([W, DIM, 16], f16, tag="ts")
                                nc.vector.tensor_tensor(prod, xs, kb,
                                                        Alu.mult)
                                prods.append(prod)
                            if len(prods) == 2:
                                nc.vector.tensor_tensor(prods[0], prods[0],
                                                        prods[1], Alu.add)
                            pending.append((prods[0], g[0][3], h))
                            # last pass: no later conv to interleave into
                            if p == 2 and gi >= 1:
                                drain_acc(1)

                    def emit_transpose(lo):
                        ktA = ktap.tile([W, 16, 32], f16, tag="ktA")
                        nc.scalar.dma_start_transpose(
                            out=ktA,
                            in_=ker_st[:, lo:lo + 16, :]
                            .rearrange("ti r q -> ti (r q)"))
                        return ktA

                    def conv2_chunk(ci):
                        g0, nr = c2[ci]
                        ps2 = ps2p.tile([18, 4, W], f32, tag="ps2")
                        for idx in range(9):
                            di, dj = divmod(idx, 3)
                            rhs = y_f[:, g0 + di - 1:g0 + di - 1 + nr,
                                      dj:dj + W]
                            nc.tensor.matmul(
                                ps2[:, :nr, :], lhsT=w2_sb[:, idx, :], rhs=rhs,
                                start=(idx == 0), stop=(idx == 8))
                        nc.scalar.activation(ker_st[0:18, g0 - 2:g0 - 2 + nr, :],
                                             ps2[:, :nr], Act.Identity,
                                             bias=b2_sb, scale=1.0)
                        if p > 0:
                            drain_acc(2 if ci % 2 == 1 else 1)

                    # interleave: conv2 chunk k only needs conv1 chunks
                    # <= k+1, so each 16-row kernel half completes (and
                    # its products start) as early as possible
                    conv1_chunk(0)
                    conv1_chunk(1)
                    for k in range(len(c2)):
                        if k + 2 < len(c1):
                            conv1_chunk(k + 2)
                        conv2_chunk(k)
                        if p < 2 and k == 3:
                            half_pipeline(p, 0, emit_transpose(0))
                    if p < 2:
                        half_pipeline(p, 1, emit_transpose(16))
                    else:
                        ktA2 = emit_transpose(0)
                        half_pipeline(2, 0, ktA2)
                        half_pipeline(2, 1, ktA2)

                # --- normalization: c = 1/45 - mean(ker); out += c * S
                # with S = 3x3 box of U.  The q-box comes from the three
                # shifted-identity accumulates; edge replication doubles
                # c at q=0/127 in the center (dj=1) tile ---
                nc.vector.tensor_tensor(r_p[0], r_p[0], r_p[1], Alu.add)
                nc.vector.tensor_tensor(r_p[0], r_p[0], r_p[2], Alu.add)
                c_sb = ktp.tile([W, SLAB], f32, name="c_sb")
                nc.vector.tensor_scalar(c_sb, r_p[0], -1.0 / 45.0, 1.0 / 45.0,
                                        Alu.mult, Alu.add)
                c_bf = ktp.tile([W, SLAB], f16, name="c_bf")
                nc.vector.tensor_copy(c_bf, c_sb)
                c_db = ktp.tile([W, SLAB], f16, name="c_db")
                nc.vector.tensor_scalar_mul(c_db, c_bf, em_sb[:, 2:3])
                c_p = ktp.tile([W, SLAB], f16, name="c_p")
                c_m = ktp.tile([W, SLAB], f16, name="c_m")
                nc.gpsimd.dma_start(out=c_p[0:127], in_=c_bf[1:128])
                nc.gpsimd.dma_start(out=c_p[127:128], in_=c_bf[127:128])
                nc.gpsimd.dma_start(out=c_m[1:128], in_=c_bf[0:127])
                nc.gpsimd.dma_start(out=c_m[0:1], in_=c_bf[0:1])
                for (cc_, djc) in ((c_db, 1), (c_p, 0), (c_m, 2)):
                    for h in range(2):
                        cs = tp.tile([W, DIM, 16], f16, tag="ts")
                        nc.vector.tensor_tensor(
                            cs, ub[:, :, 16 * h:16 * h + 16],
                            cc_[:, 16 * h:16 * h + 16].unsqueeze(1)
                            .broadcast_to((W, DIM, 16)),
                            Alu.mult)
                        pending.append((cs, djc, h))
                        drain_acc(1)

                final_drain()

                # drain acc -> fp16, then DMA-xbar transposes to DRAM
                acc_sb = ktp.tile([W, DIM * SLAB], f16, name="acc_sb")
                for cc in range(2):
                    sl = slice(512 * cc, 512 * (cc + 1))
                    nc.scalar.activation(acc_sb[:, sl], acc[:, sl],
                                         Act.Copy, scale=1.0)
                for cc in range(2, 4):
                    sl = slice(512 * cc, 512 * (cc + 1))
                    nc.vector.tensor_copy(acc_sb[:, sl], acc[:, sl])
                for h in range(2):
                    ob = obp.tile([128, 8, W], f16, tag="ob")
                    nc.sync.dma_start_transpose(
                        out=ob, in_=acc_sb[:, 1024 * h:1024 * (h + 1)])
                    nc.sync.dma_start(out=out_d[:, 8 * h:8 * h + 8, :],
                                      in_=ob)

    return nc


def _get_program():
    if "nc" not in _PROGRAM_CACHE:
        nc = _build_program()
        nc.finalize()
        _PROGRAM_CACHE["nc"] = nc
    return _PROGRAM_CACHE["nc"]


def _host_prep(x, w1, b1, w2, b2):
    """Build the 8 per-core input maps from full inputs."""
    x = np.asarray(x, dtype=np.float32)
    w1 = np.asarray(w1, dtype=np.float32)
    b1 = np.asarray(b1, dtype=np.float32)
    w2 = np.asarray(w2, dtype=np.float32)
    b2 = np.asarray(b2, dtype=np.float32)
    f16 = np.float16

    # block-diagonal packed weights: passes 0/1 = 2 frames, pass 2 = the
    # two spatial halves of frame 4 -> identical weight matrices
    w1t = w1.transpose(1, 2, 3, 0).reshape(DIM, 9, DIM)   # [ci, tap, o]
    w2t = w2.transpose(1, 2, 3, 0).reshape(DIM, 9, 9)
    w1a = np.zeros((128, 9, 128), np.float32)
    w1a[0:64, :, 0:64] = w1t
    w1a[64:128, :, 64:128] = w1t
    w2a = np.zeros((128, 9, 18), np.float32)
    w2a[0:64, :, 0:9] = w2t
    w2a[64:128, :, 9:18] = w2t

    b1r = np.concatenate([b1, b1]).reshape(128, 1).astype(np.float32)
    b2r = np.concatenate([b2, b2]).reshape(18, 1).astype(np.float32)
    idf = np.eye(128, dtype=f16)
    w1a = w1a.astype(f16)
    w2a = w2a.astype(f16)

    in_maps = []
    for core in range(NCORES):
        b, s = divmod(core, 4)
        r0 = s * SLAB
        # passes 0/1 conv input: frames (2p, 2p+1) on the partition
        # halves, x rows r0-2 .. r0+33 zero padded, cols -1..128 zero
        xc01 = np.zeros((2, 128, GH, GW), np.float32)
        lo = max(0, r0 - 2)
        hi = min(H, r0 + 34)
        for p in range(2):
            for f in range(2):
                t = 2 * p + f
                xc01[p, f * 64:(f + 1) * 64,
                     lo - (r0 - 2):hi - (r0 - 2), 1:129] = x[b, :, t, lo:hi, :]
        # pass 2: frame 4 split into two 16-row halves on the partition
        # halves (plus conv halo)
        xc2 = np.zeros((128, GH2, GW), np.float32)
        for h2 in range(2):
            bx = r0 - 2 if h2 == 0 else r0 + 14
            lo2 = max(0, bx)
            hi2 = min(H, bx + GH2)
            xc2[h2 * 64:(h2 + 1) * 64, lo2 - bx:hi2 - bx, 1:129] = \
                x[b, :, 4, lo2:hi2, :]
        # filter input, pixel-partition, center (dj=1) copy only; xts =
        # the r0-based row window so di=1 product slices start 4B-aligned
        rows = np.clip(np.arange(r0 - 1, r0 + 33), 0, H - 1)
        xt = x[b][:, :, rows, :].transpose(3, 1, 0, 2)          # (w,t,c,34)
        xts = x[b][:, :, r0:r0 + 32, :].transpose(3, 1, 0, 2)   # (w,t,c,32)
        # shifted identities for the dj=0/dj=2 accumulates
        sm = np.zeros((128, 2, 128), np.float32)
        sm[0:127, 0, :] = np.eye(128, dtype=np.float32)[1:128]   # m = p+1
        sm[1:128, 1, :] = np.eye(128, dtype=np.float32)[0:127]   # m = p-1
        em = np.zeros((128, 3), np.float32)
        em[0, 0] = 1.0      # q=0 edge (dj=0 term folds into dj=1)
        em[127, 1] = 1.0    # q=127 edge (dj=2 term folds into dj=1)
        em[:, 2] = 1.0      # edge-doubling mask for the c*S center tile
        em[0, 2] = 2.0
        em[127, 2] = 2.0
        # conv2 zero-pad masks for y rows outside the image
        ym = np.ones((128, 2), np.float32)
        if s == 0:
            ym[:, 0] = 0.0
        if s == 3:
            ym[:, 1] = 0.0
        ym2 = np.ones((128, 2), np.float32)
        if s == 0:
            ym2[0:64, 0] = 0.0
        if s == 3:
            ym2[64:128, 1] = 0.0
        in_maps.append({
            "xc01": xc01.astype(f16), "xc2": xc2.astype(f16),
            "xt": xt.astype(f16), "xts": xts.astype(f16),
            "w1": w1a, "w2": w2a, "b1r": b1r, "b2r": b2r,
            "ym": ym, "ym2": ym2, "idf": idf, "sm": sm.astype(f16),
            "em": em,
        })
    return in_maps


def kernel(x, w1, b1, w2, b2):
    from concourse.bass_utils import run_bass_kernel_spmd

    nc = _get_program()
    in_maps = _host_prep(x, w1, b1, w2, b2)
    res = run_bass_kernel_spmd(nc, in_maps, list(range(NCORES)))
    out = np.zeros((2, DIM, H, W), dtype=np.float32)
    for core in range(NCORES):
        b, s = divmod(core, 4)
        # device layout: o[b_, a, q] = acc[q, flat = 128*a + b_] with
        # flat = 1024*h + 16*c + rr and out row r = 16*h + rr
        o = res.results[core]["out"].astype(np.float32)
        o = o.transpose(1, 0, 2).reshape(2, DIM, 16, W)
        o = o.transpose(1, 0, 2, 3).reshape(DIM, SLAB, W)
        out[b, :, s * SLAB:(s + 1) * SLAB, :] = o
    return out


# revision 68
# speedup vs baseline: 1.1503x; 1.0225x over previous
"""Trainium2 Bass kernel for nn_DynamicFiltering (v2).

Computation (per batch b):
  y  = LeakyReLU(conv2d(x_t, w1, b1), 0.2)        per frame t
  ker = conv2d(y, w2, b2)                          (t, 9, h, w)
  ker = ker - mean_K(ker) + 1/45                   per-pixel over K = 45
  out[c,h,w] = sum_{t,k1,k2} x_edge[c,t,h+k1-1,w+k2-1] * ker[t,k1,k2][h,w]

Sharding: 8 cores = 2 batches x 4 H-slabs of 32 rows.

Structure (v2, vs the 234us bf16 baseline):
  - all 16-bit data is fp16 (same speed as bf16, ~8x less quant error)
  - conv1 leaky relu is a single Act Prelu(alpha=0.2) drain (verified on
    HW: Prelu honors alpha, Lrelu hardcodes 0.01) -> GpSimd fully freed
  - pass 2 (lone frame 4) is spatially halved: both partition halves
    carry frame-4 channels over half the rows, same block-diag weights
    as the 2-frame passes -> conv1/conv2 pass-2 matmul cycles halve
  - dynamic-filter products run on DVE in 2x fp16 mode: the di=1 row
    window is served by a separate host copy (xts) so every slice start
    is 4B-aligned
  - only the CENTER (dj=1) pixel-partition x copy is loaded; the dj
    column shift moves to the kernel side: kt2 is partition-shifted by
    +-1 via SBUF-SBUF DMA (2KB/partition vs 40KB for x copies) and the
    PE accumulate uses shifted identity matrices (eye(k=+-1)).  The
    edge-replication terms (q=0 dj=0, q=127 dj=2) multiply the same x
    element as the dj=1 term, so they fold into the dj=1 kernel's edge
    values with one tiny DVE add per edge per pass
  - no tree reduction / pass sums: product pairs (same dj) are added
    once on DVE, then PE matmuls accumulate each pair tile into a
    persistent 4-bank PSUM accumulator, interleaved with the next
    pass's conv matmuls (PSUM: 2 conv1 + 2 conv2 + 4 acc banks = 8)
  - normalization term c*S as in v1 (U chain: sums on DVE in the
    pre-product idle window, shifts/box-sums on GpSimd)
  - output: acc -> fp16 -> 16 DMA-xbar transposes -> DRAM (host casts
    to fp32); no PE transposes, no fp32 identity
  - startup: w1 + pass-0 conv input bands issued first on the sync
    HWDGE ring; all other constants + pass-1/2 inputs on the act ring
"""

import numpy as np

DIM = 64
T = 5
H = 128
W = 128
SLAB = 32          # output rows per core
NCORES = 8
GH = 36            # conv grid rows, passes 0/1: slab + 2*2 halo
GH2 = 20           # conv grid rows, pass 2 halves: 16 + 2*2 halo
GW = 130           # conv grid cols: W + 2
FR = 34            # filter rows: slab + 2 halo
NPASS = 3

_PROGRAM_CACHE = {}

C1_CHUNKS = [(1 + 4 * i, 4) for i in range(8)] + [(33, 2)]
C2_CHUNKS = [(2 + 4 * i, 4) for i in range(8)]
C1_CHUNKS2 = [(1 + 4 * i, 4) for i in range(4)] + [(17, 2)]
C2_CHUNKS2 = [(2 + 4 * i, 4) for i in range(4)]


def _build_program():
    import concourse.bacc as bacc
    import concourse.mybir as mybir
    from concourse.tile import TileContext

    f32 = mybir.dt.float32
    f16 = mybir.dt.float16
    u16 = mybir.dt.uint16
    Act = mybir.ActivationFunctionType
    Alu = mybir.AluOpType
    Ax = mybir.AxisListType

    nc = bacc.Bacc("TRN2", debug=False)

    xc01_d = nc.dram_tensor("xc01", [2, 128, GH, GW], f16, kind="ExternalInput").ap()
    xc2_d = nc.dram_tensor("xc2", [128, GH2, GW], f16, kind="ExternalInput").ap()
    xt_d = nc.dram_tensor("xt", [W, T, DIM, FR], f16, kind="ExternalInput").ap()
    xts_d = nc.dram_tensor("xts", [W, T, DIM, SLAB], f16, kind="ExternalInput").ap()
    sm_d = nc.dram_tensor("sm", [128, 2, 128], f16, kind="ExternalInput").ap()
    em_d = nc.dram_tensor("em", [128, 3], f32, kind="ExternalInput").ap()
    w1_d = nc.dram_tensor("w1", [128, 9, 128], f16, kind="ExternalInput").ap()
    w2_d = nc.dram_tensor("w2", [128, 9, 18], f16, kind="ExternalInput").ap()
    b1_d = nc.dram_tensor("b1r", [128, 1], f32, kind="ExternalInput").ap()
    b2_d = nc.dram_tensor("b2r", [18, 1], f32, kind="ExternalInput").ap()
    ym_d = nc.dram_tensor("ym", [128, 2], f32, kind="ExternalInput").ap()
    ym2_d = nc.dram_tensor("ym2", [128, 2], f32, kind="ExternalInput").ap()
    idf_d = nc.dram_tensor("idf", [128, 128], f16, kind="ExternalInput").ap()
    # out[b_, a, q] = acc[q, 128*a + b_]; the host unscrambles (c r) =
    # 128*a + b_ back to [c, r] (one contiguous DMA instead of 16)
    out_d = nc.dram_tensor("out", [128, 16, W], f16, kind="ExternalOutput").ap()

    with TileContext(nc) as tc:
        with (
            tc.tile_pool(name="consts", bufs=1) as cpool,
            tc.tile_pool(name="xtp", bufs=1) as xtp,
            tc.tile_pool(name="xcp", bufs=2) as xcp,
            tc.tile_pool(name="xc2p", bufs=1) as xc2p,
            tc.tile_pool(name="yp", bufs=3) as yp,
            tc.tile_pool(name="ksh", bufs=2) as kshp,
            tc.tile_pool(name="kst", bufs=1) as kstp,
            tc.tile_pool(name="kta", bufs=2) as ktap,
            tc.tile_pool(name="ktp", bufs=1) as ktp,
            tc.tile_pool(name="up", bufs=1) as up,
            tc.tile_pool(name="tp", bufs=18) as tp,
            tc.tile_pool(name="obp", bufs=2) as obp,
        ):
            # ---- startup DMAs: sync ring carries only what gates the ----
            # ---- first conv1 matmuls (w1 + pass-0 input bands)        ----
            w1_sb = cpool.tile([128, 9, 128], f16)
            nc.sync.dma_start(out=w1_sb, in_=w1_d)

            def load_xc01(p, eng):
                t = xcp.tile([128, GH, GW], f16, tag="xc")
                for r0b, r1b in ((0, 8), (8, 16), (16, 24), (24, 32), (32, 36)):
                    eng.dma_start(out=t[:, r0b:r1b], in_=xc01_d[p, :, r0b:r1b])
                return t

            xc_p0 = load_xc01(0, nc.sync)

            # filter inputs on the sync HWDGE ring AFTER the pass-0 conv
            # bands: per-ring FIFO means the bands drain at full
            # bandwidth first (on the SWDGE ring their huge descriptors
            # monopolized the SDMA engines and stalled conv1 pass 0)
            xt1 = xtp.tile([W, T, DIM, FR], f16, name="xt1")
            xts1 = xtp.tile([W, T, DIM, SLAB], f16, name="xts1")
            nc.sync.dma_start(out=xt1, in_=xt_d)
            nc.sync.dma_start(out=xts1, in_=xts_d)

            # act HWDGE ring: everything else, in need-order
            b1_sb = cpool.tile([128, 1], f32)
            nc.scalar.dma_start(out=b1_sb, in_=b1_d)
            w2_sb = cpool.tile([128, 9, 18], f16)
            nc.scalar.dma_start(out=w2_sb, in_=w2_d)
            b2_sb = cpool.tile([18, 1], f32)
            nc.scalar.dma_start(out=b2_sb, in_=b2_d)
            ym_sb = cpool.tile([128, 2], f32)
            nc.scalar.dma_start(out=ym_sb, in_=ym_d)
            ym2_sb = cpool.tile([128, 2], f32)
            nc.scalar.dma_start(out=ym2_sb, in_=ym2_d)
            idf_sb = cpool.tile([128, 128], f16)
            nc.scalar.dma_start(out=idf_sb, in_=idf_d)
            sm_sb = cpool.tile([128, 2, 128], f16)
            nc.scalar.dma_start(out=sm_sb, in_=sm_d)
            em_sb = cpool.tile([128, 3], f32)
            nc.scalar.dma_start(out=em_sb, in_=em_d)
            # pass-1/2 conv inputs also on the sync ring: DMAs on the act
            # ring would block the conv1 Prelu drains behind their
            # completions (act-queue FIFO) and stall conv1 on PSUM reuse
            xc_p1 = load_xc01(1, nc.sync)
            xc_p2 = xc2p.tile([128, GH2, GW], f16)
            for r0b, r1b in ((0, 8), (8, 16), (16, 20)):
                nc.sync.dma_start(out=xc_p2[:, r0b:r1b], in_=xc2_d[:, r0b:r1b])

            # conv2 -> kernel staging (ti on partitions)
            ker_st = kstp.tile([32, SLAB, W], f16)
            nc.gpsimd.memset(ker_st.bitcast(u16), 0)
            # warm the Q7 tensor-op ucode (~6us LIBRARY_RELOAD) off the
            # critical path; the writes are zeros into staging rows that
            # conv2 either overwrites or the transpose reads as zero
            nc.gpsimd.tensor_copy(ker_st[:, 0:1, 0:64], ker_st[:, 0:1, 64:128])
            nc.gpsimd.tensor_tensor(ker_st[:, 0:1, 0:64], ker_st[:, 0:1, 0:64],
                                    ker_st[:, 0:1, 64:128], Alu.add)

            # y tiles pre-allocated; edge cols zeroed up front on gpsimd
            y_t = [yp.tile([128, GH, GW], f16, name=f"y{p}", tag="y")
                   for p in range(3)]
            for p in range(3):
                nr = 34 if p < 2 else 18
                nc.gpsimd.memset(y_t[p][:, 1:1 + nr, 0:1].bitcast(u16), 0)
                nc.gpsimd.memset(y_t[p][:, 1:1 + nr, 129:130].bitcast(u16), 0)

            # per-pixel kernels, half-major: kt2[p][q, h, ti, r] with
            # h = 16-row kernel half (enables per-half pipelining and
            # contiguous shift DMAs)
            kt2 = [ktp.tile([W, 2, 18, 16], f16, name=f"kt2_{p}")
                   for p in range(NPASS)]

            # U chain on DVE in the idle window before the first products
            # (xt arrives ~30us, first kernels ~50us).  ub = 3-row box sum
            # of U = sum_t x_t; the q-direction box happens at accumulate
            # time via the same shifted-identity trick as the products,
            # with shifted copies of the tiny coefficient c instead of a
            # (slow) partition-shifted copy of U
            u_c = up.tile([W, DIM, FR], f16, name="u_c")
            ub = up.tile([W, DIM, SLAB], f16, name="ub")
            nc.vector.tensor_tensor(u_c, xt1[:, 0], xt1[:, 1], Alu.add)
            for t_i in (2, 3, 4):
                nc.vector.tensor_tensor(u_c, u_c, xt1[:, t_i], Alu.add)
            nc.vector.tensor_tensor(ub, u_c[:, :, 0:SLAB],
                                    u_c[:, :, 1:SLAB + 1], Alu.add)
            nc.vector.tensor_tensor(ub, ub, u_c[:, :, 2:SLAB + 2], Alu.add)

            r_p = [ktp.tile([W, SLAB], f32, name=f"r{p}") for p in range(NPASS)]

            with (
                tc.tile_pool(name="ps1", bufs=2, space="PSUM") as ps1p,
                tc.tile_pool(name="ps2", bufs=2, space="PSUM") as ps2p,
                tc.tile_pool(name="acc", bufs=1, space="PSUM") as accp,
            ):
                # acc layout [q, (h, c, rr)]: half-major so a half tile
                # accumulates into its own 2 PSUM banks
                # PE warm-up: dummy matmuls as soon as the weights land
                # keep the HAM clock gate at full rate for the first real
                # conv chunks (results never read; finite garbage)
                wps = ps2p.tile([18, 4, W], f32, tag="ps2")
                for _ in range(24):
                    nc.tensor.matmul(wps[:, 0, 0:64], lhsT=w2_sb[:, 0, :],
                                     rhs=w1_sb[:, 0, 0:64],
                                     start=True, stop=True)

                acc = accp.tile([W, DIM * SLAB], f32)
                pending = []          # (tile, dj, h) awaiting accumulate
                acc_first = [True, True]

                def acc_mm(tile, dj, h, last):
                    # dj=1: plain identity; dj=0/2: shifted identity
                    # applies the +-1 pixel-column shift of the patches
                    lhs = (sm_sb[:, 0, :], idf_sb, sm_sb[:, 1, :])[dj]
                    fl = tile.rearrange("q c r -> q (c r)")
                    for cc in range(2):
                        sl = slice(1024 * h + 512 * cc,
                                   1024 * h + 512 * (cc + 1))
                        nc.tensor.matmul(acc[:, sl], lhsT=lhs,
                                         rhs=fl[:, 512 * cc:512 * (cc + 1)],
                                         start=acc_first[h], stop=last)
                    acc_first[h] = False

                def drain_acc(n):
                    for _ in range(min(n, len(pending))):
                        tile, dj, h = pending.pop(0)
                        acc_mm(tile, dj, h, False)

                def final_drain():
                    last_of = {}
                    for i, (_, _, h) in enumerate(pending):
                        last_of[h] = i
                    for i, (tile, dj, h) in enumerate(pending):
                        acc_mm(tile, dj, h, last_of[h] == i)
                    pending.clear()

                for p in range(NPASS):
                    xc_f = (xc_p0, xc_p1, xc_p2)[p]
                    y_f = y_t[p]
                    c1 = C1_CHUNKS if p < 2 else C1_CHUNKS2
                    c2 = C2_CHUNKS if p < 2 else C2_CHUNKS2

                    ymm = ym_sb if p < 2 else ym2_sb
                    hrow = 34 if p < 2 else 18

                    def conv1_chunk(ci):
                        g0, nr = c1[ci]
                        ps = ps1p.tile([128, 4, W], f32, tag="ps1")
                        for idx in range(9):
                            di, dj = divmod(idx, 3)
                            rhs = xc_f[:, g0 + di - 1:g0 + di - 1 + nr,
                                       dj:dj + W]
                            nc.tensor.matmul(
                                ps[:, :nr, :], lhsT=w1_sb[:, idx, :], rhs=rhs,
                                start=(idx == 0), stop=(idx == 8))
                        nc.scalar.activation(y_f[:, g0:g0 + nr, 1:129],
                                             ps[:, :nr], Act.Prelu,
                                             bias=b1_sb, scale=1.0, alpha=0.2)
                        # conv2 zero-pads rows outside the image: kill the
                        # y halo row as soon as its chunk drains
                        if ci == 0:
                            nc.scalar.activation(y_f[:, 1:2, 1:129],
                                                 y_f[:, 1:2, 1:129],
                                                 Act.Copy, scale=ymm[:, 0:1])
                        if ci == len(c1) - 1:
                            nc.scalar.activation(
                                y_f[:, hrow:hrow + 1, 1:129],
                                y_f[:, hrow:hrow + 1, 1:129],
                                Act.Copy, scale=ymm[:, 1:2])
                        if p >= 1 and ci >= 1:
                            drain_acc(1)

                    def half_pipeline(p, h, ktA):
                        # repack ktA[q, r, ti] -> kt2[p][q, h, ti, r] on
                        # DVE (288 elems -> ~0.4us, and no cross-engine
                        # latency before the products that follow)
                        if p < 2:
                            nc.vector.tensor_copy(
                                kt2[p][:, h, :, :],
                                ktA[:, :, 0:18].rearrange("q r t -> q t r"))
                        else:
                            # spatial halves arrive as taps 0-8 / 9-17
                            nc.vector.tensor_copy(
                                kt2[2][:, h, 0:9, :],
                                ktA[:, :, 9 * h:9 * h + 9]
                                .rearrange("q r t -> q t r"))
                        nt = 18 if p < 2 else 9
                        # kernel sums for the normalization coefficient
                        # (must read the PRE-merge values)
                        nc.vector.tensor_reduce(
                            r_p[p][:, 16 * h:16 * h + 16],
                            kt2[p][:, h, 0:nt, :].rearrange("q t r -> q r t"),
                            axis=Ax.X, op=Alu.add)
                        # fold the edge-replicated dj=0 (q=0) / dj=2
                        # (q=127) terms into the dj=1 kernel: they multiply
                        # the same x element as the dj=1 term there.
                        # Engines can't start mid-partition, so mask with
                        # a per-partition one-hot
                        ev = kt2[p][:, h, 0:nt, :].rearrange(
                            "q (a b) r -> q a b r", b=3)
                        nc.vector.scalar_tensor_tensor(
                            ev[:, :, 1, :], ev[:, :, 0, :], em_sb[:, 0:1],
                            ev[:, :, 1, :], Alu.mult, Alu.add)
                        nc.vector.scalar_tensor_tensor(
                            ev[:, :, 1, :], ev[:, :, 2, :], em_sb[:, 1:2],
                            ev[:, :, 1, :], Alu.mult, Alu.add)
                        # partition-shifted kernel copies for dj=0 / dj=2
                        # (products run at the source pixel; the PE
                        # accumulate shifts them into place)
                        # (rows ktp_t[127] / ktm_t[0] are killed by the
                        # zero row of the shifted identities, but must
                        # hold FINITE values: 0 * NaN would poison PSUM)
                        ktp_t = kshp.tile([W, 18, 16], f16, tag="kp")
                        ktm_t = kshp.tile([W, 18, 16], f16, tag="km")
                        nc.gpsimd.dma_start(out=ktp_t[0:127],
                                            in_=kt2[p][1:128, h, :, :])
                        nc.gpsimd.dma_start(out=ktp_t[127:128],
                                            in_=kt2[p][127:128, h, :, :])
                        nc.gpsimd.dma_start(out=ktm_t[1:128],
                                            in_=kt2[p][0:127, h, :, :])
                        nc.gpsimd.dma_start(out=ktm_t[0:1],
                                            in_=kt2[p][0:1, h, :, :])

                        # products: pairs (same dj) -> one DVE add -> PE
                        # accumulate (drained interleaved with conv)
                        # di order (0,2,1): the di=1 groups read xts1,
                        # which lands last on the sync ring
                        if p < 2:
                            groups = [[(2 * p + fi, fi * 9 + 3 * di + dj,
                                        di, dj) for fi in (0, 1)]
                                      for dj in (1, 0, 2) for di in (0, 2, 1)]
                        else:
                            # last pass: no pairing — each product goes
                            # straight to the PE accumulator, shortening
                            # the serial DVE tail (the extra MMs land in
                            # the PE's end-game idle gaps)
                            groups = [[(4, 3 * di + dj, di, dj)]
                                      for dj in (1, 0, 2) for di in (0, 2, 1)]
                        for gi, g in enumerate(groups):
                            prods = []
                            for (f, ti, di, dj) in g:
                                if dj == 1:
                                    kb = kt2[p][:, h, ti, :]
                                else:
                                    kb = (ktp_t if dj == 0 else ktm_t)[:, ti, :]
                                kb = kb.unsqueeze(1)\
                                    .broadcast_to((W, DIM, 16))
                                if di == 1:
                                    xs = xts1[:, f, :, 16 * h:16 * h + 16]
                                else:
                                    xs = xt1[:, f, :,
                                             di + 16 * h:di + 16 * h + 16]
                                prod = tp.tile([W, DIM, 16], f16, tag="ts")
                                nc.vector.tensor_tensor(prod, xs, kb,
                                                        Alu.mult)
                                prods.append(prod)
                            if len(prods) == 2:
                                nc.vector.tensor_tensor(prods[0], prods[0],
                                                        prods[1], Alu.add)
                            pending.append((prods[0], g[0][3], h))
                            # last pass: no later conv to interleave into
                            if p == 2 and gi >= 1:
                                drain_acc(1)

                    def emit_transpose(lo):
                        ktA = ktap.tile([W, 16, 32], f16, tag="ktA")
                        nc.scalar.dma_start_transpose(
                            out=ktA,
                            in_=ker_st[:, lo:lo + 16, :]
                            .rearrange("ti r q -> ti (r q)"))
                        return ktA

                    def conv2_chunk(ci):
                        g0, nr = c2[ci]
                        ps2 = ps2p.tile([18, 4, W], f32, tag="ps2")
                        for idx in range(9):
                            di, dj = divmod(idx, 3)
                            rhs = y_f[:, g0 + di - 1:g0 + di - 1 + nr,
                                      dj:dj + W]
                            nc.tensor.matmul(
                                ps2[:, :nr, :], lhsT=w2_sb[:, idx, :], rhs=rhs,
                                start=(idx == 0), stop=(idx == 8))
                        nc.scalar.activation(ker_st[0:18, g0 - 2:g0 - 2 + nr, :],
                                             ps2[:, :nr], Act.Identity,
                                             bias=b2_sb, scale=1.0)
                        if p > 0:
                            drain_acc(2 if ci % 2 == 1 else 1)

                    # interleave: conv2 chunk k only needs conv1 chunks
                    # <= k+1, so each 16-row kernel half completes (and
                    # its products start) as early as possible
                    conv1_chunk(0)
                    conv1_chunk(1)
                    for k in range(len(c2)):
                        if k + 2 < len(c1):
                            conv1_chunk(k + 2)
                        conv2_chunk(k)
                        if p < 2 and k == 3:
                            half_pipeline(p, 0, emit_transpose(0))
                    if p < 2:
                        half_pipeline(p, 1, emit_transpose(16))
                    else:
                        ktA2 = emit_transpose(0)
                        half_pipeline(2, 0, ktA2)
                        half_pipeline(2, 1, ktA2)

                # --- normalization: c = 1/45 - mean(ker); out += c * S
                # with S = 3x3 box of U.  The q-box comes from the three
                # shifted-identity accumulates; edge replication doubles
                # c at q=0/127 in the center (dj=1) tile ---
                nc.vector.tensor_tensor(r_p[0], r_p[0], r_p[1], Alu.add)
                nc.vector.tensor_tensor(r_p[0], r_p[0], r_p[2], Alu.add)
                c_sb = ktp.tile([W, SLAB], f32, name="c_sb")
                nc.vector.tensor_scalar(c_sb, r_p[0], -1.0 / 45.0, 1.0 / 45.0,
                                        Alu.mult, Alu.add)
                c_bf = ktp.tile([W, SLAB], f16, name="c_bf")
                nc.vector.tensor_copy(c_bf, c_sb)
                c_db = ktp.tile([W, SLAB], f16, name="c_db")
                nc.vector.tensor_scalar_mul(c_db, c_bf, em_sb[:, 2:3])
                c_p = ktp.tile([W, SLAB], f16, name="c_p")
                c_m = ktp.tile([W, SLAB], f16, name="c_m")
                nc.gpsimd.dma_start(out=c_p[0:127], in_=c_bf[1:128])
                nc.gpsimd.dma_start(out=c_p[127:128], in_=c_bf[127:128])
                nc.gpsimd.dma_start(out=c_m[1:128], in_=c_bf[0:127])
                nc.gpsimd.dma_start(out=c_m[0:1], in_=c_bf[0:1])
                # half 0 first: its PSUM banks stop, drain and ship while
                # the DVE still produces half 1
                for h in range(2):
                    for (cc_, djc) in ((c_db, 1), (c_p, 0), (c_m, 2)):
                        cs = tp.tile([W, DIM, 16], f16, tag="ts")
                        nc.vector.tensor_tensor(
                            cs, ub[:, :, 16 * h:16 * h + 16],
                            cc_[:, 16 * h:16 * h + 16].unsqueeze(1)
                            .broadcast_to((W, DIM, 16)),
                            Alu.mult)
                        pending.append((cs, djc, h))
                        drain_acc(1)

                final_drain()

                # drain acc -> fp16, then DMA-xbar transposes to DRAM
                acc_sb = ktp.tile([W, DIM * SLAB], f16, name="acc_sb")
                for cc in range(2):
                    sl = slice(512 * cc, 512 * (cc + 1))
                    nc.scalar.activation(acc_sb[:, sl], acc[:, sl],
                                         Act.Copy, scale=1.0)
                for cc in range(2, 4):
                    sl = slice(512 * cc, 512 * (cc + 1))
                    nc.vector.tensor_copy(acc_sb[:, sl], acc[:, sl])
                for h in range(2):
                    ob = obp.tile([128, 8, W], f16, tag="ob")
                    nc.sync.dma_start_transpose(
                        out=ob, in_=acc_sb[:, 1024 * h:1024 * (h + 1)])
                    nc.sync.dma_start(out=out_d[:, 8 * h:8 * h + 8, :],
                                      in_=ob)

    return nc


def _get_program():
    if "nc" not in _PROGRAM_CACHE:
        nc = _build_program()
        nc.finalize()
        _PROGRAM_CACHE["nc"] = nc
    return _PROGRAM_CACHE["nc"]


def _host_prep(x, w1, b1, w2, b2):
    """Build the 8 per-core input maps from full inputs."""
    x = np.asarray(x, dtype=np.float32)
    w1 = np.asarray(w1, dtype=np.float32)
    b1 = np.asarray(b1, dtype=np.float32)
    w2 = np.asarray(w2, dtype=np.float32)
    b2 = np.asarray(b2, dtype=np.float32)
    f16 = np.float16

    # block-diagonal packed weights: passes 0/1 = 2 frames, pass 2 = the
    # two spatial halves of frame 4 -> identical weight matrices
    w1t = w1.transpose(1, 2, 3, 0).reshape(DIM, 9, DIM)   # [ci, tap, o]
    w2t = w2.transpose(1, 2, 3, 0).reshape(DIM, 9, 9)
    w1a = np.zeros((128, 9, 128), np.float32)
    w1a[0:64, :, 0:64] = w1t
    w1a[64:128, :, 64:128] = w1t
    w2a = np.zeros((128, 9, 18), np.float32)
    w2a[0:64, :, 0:9] = w2t
    w2a[64:128, :, 9:18] = w2t

    b1r = np.concatenate([b1, b1]).reshape(128, 1).astype(np.float32)
    b2r = np.concatenate([b2, b2]).reshape(18, 1).astype(np.float32)
    idf = np.eye(128, dtype=f16)
    w1a = w1a.astype(f16)
    w2a = w2a.astype(f16)

    in_maps = []
    for core in range(NCORES):
        b, s = divmod(core, 4)
        r0 = s * SLAB
        # passes 0/1 conv input: frames (2p, 2p+1) on the partition
        # halves, x rows r0-2 .. r0+33 zero padded, cols -1..128 zero
        xc01 = np.zeros((2, 128, GH, GW), np.float32)
        lo = max(0, r0 - 2)
        hi = min(H, r0 + 34)
        for p in range(2):
            for f in range(2):
                t = 2 * p + f
                xc01[p, f * 64:(f + 1) * 64,
                     lo - (r0 - 2):hi - (r0 - 2), 1:129] = x[b, :, t, lo:hi, :]
        # pass 2: frame 4 split into two 16-row halves on the partition
        # halves (plus conv halo)
        xc2 = np.zeros((128, GH2, GW), np.float32)
        for h2 in range(2):
            bx = r0 - 2 if h2 == 0 else r0 + 14
            lo2 = max(0, bx)
            hi2 = min(H, bx + GH2)
            xc2[h2 * 64:(h2 + 1) * 64, lo2 - bx:hi2 - bx, 1:129] = \
                x[b, :, 4, lo2:hi2, :]
        # filter input, pixel-partition, center (dj=1) copy only; xts =
        # the r0-based row window so di=1 product slices start 4B-aligned
        rows = np.clip(np.arange(r0 - 1, r0 + 33), 0, H - 1)
        xt = x[b][:, :, rows, :].transpose(3, 1, 0, 2)          # (w,t,c,34)
        xts = x[b][:, :, r0:r0 + 32, :].transpose(3, 1, 0, 2)   # (w,t,c,32)
        # shifted identities for the dj=0/dj=2 accumulates
        sm = np.zeros((128, 2, 128), np.float32)
        sm[0:127, 0, :] = np.eye(128, dtype=np.float32)[1:128]   # m = p+1
        sm[1:128, 1, :] = np.eye(128, dtype=np.float32)[0:127]   # m = p-1
        em = np.zeros((128, 3), np.float32)
        em[0, 0] = 1.0      # q=0 edge (dj=0 term folds into dj=1)
        em[127, 1] = 1.0    # q=127 edge (dj=2 term folds into dj=1)
        em[:, 2] = 1.0      # edge-doubling mask for the c*S center tile
        em[0, 2] = 2.0
        em[127, 2] = 2.0
        # conv2 zero-pad masks for y rows outside the image
        ym = np.ones((128, 2), np.float32)
        if s == 0:
            ym[:, 0] = 0.0
        if s == 3:
            ym[:, 1] = 0.0
        ym2 = np.ones((128, 2), np.float32)
        if s == 0:
            ym2[0:64, 0] = 0.0
        if s == 3:
            ym2[64:128, 1] = 0.0
        in_maps.append({
            "xc01": xc01.astype(f16), "xc2": xc2.astype(f16),
            "xt": xt.astype(f16), "xts": xts.astype(f16),
            "w1": w1a, "w2": w2a, "b1r": b1r, "b2r": b2r,
            "ym": ym, "ym2": ym2, "idf": idf, "sm": sm.astype(f16),
            "em": em,
        })
    return in_maps


def kernel(x, w1, b1, w2, b2):
    from concourse.bass_utils import run_bass_kernel_spmd

    nc = _get_program()
    in_maps = _host_prep(x, w1, b1, w2, b2)
    res = run_bass_kernel_spmd(nc, in_maps, list(range(NCORES)))
    out = np.zeros((2, DIM, H, W), dtype=np.float32)
    for core in range(NCORES):
        b, s = divmod(core, 4)
        # device layout: o[b_, a, q] = acc[q, flat = 128*a + b_] with
        # flat = 1024*h + 16*c + rr and out row r = 16*h + rr
        o = res.results[core]["out"].astype(np.float32)
        o = o.transpose(1, 0, 2).reshape(2, DIM, 16, W)
        o = o.transpose(1, 0, 2, 3).reshape(DIM, SLAB, W)
        out[b, :, s * SLAB:(s + 1) * SLAB, :] = o
    return out
